# revision 37
# baseline (speedup 1.0000x reference)
"""Trainium2 Bass kernel for nn_CMAF (cross-modal attention fusion block).

Layout: feature-major activations on-chip — every tile is
[128 features (partitions) x 1024 samples (free)], so all matmuls are
weight-stationary bf16 with the batch as the moving free dimension.
Inputs are pre-transposed host-side into feature-major HBM layouts, so
device DMA is fully contiguous (no DMA-transpose).

Engine-balance design (ACT/DVE were the baseline bottleneck):
 - LN stats (sum of squares) for all 3 branches land in ONE [3,1024]
   PSUM tile; Ln+Exp (rsqrt) run once per LN stage on that compact tile
   instead of per-branch full tiles; per-branch ones-matmuls broadcast
   the result back to 128 partitions (PE pump is cheap).
 - Residual adds (u = o + P, x2p = f + x1) are folded into the PE as
   identity-matrix accumulation matmuls, killing 1x-rate STT DVE ops.
 - Wo@v1 is folded host-side into Wov = (C Wo) Wv and accumulated into
   the same PSUM as Wo@tp, killing the tpv add.
 - The 2-way attention softmax collapses to division by (1+exp(-d/sqrt(dh)))
   done as a single DVE tensor_tensor divide straight from PSUM.
 - Gelu ACT ops are clustered at alternating head/tail of the pipeline
   tick so the ACT table set (gelu vs natural_log_exp) switches once per
   block on average instead of twice.

Data parallel over 8 NeuronCores: 8192 samples each.
"""

import numpy as np
import ml_dtypes

import concourse.bass as bass
import concourse.mybir as mybir
from concourse.tile import TileContext
from concourse.vector_clock import ScopedClock
from concourse.bass_utils import run_bass_kernel_spmd

F32 = mybir.dt.float32
BF16 = mybir.dt.bfloat16
AL = mybir.AluOpType
AF = mybir.ActivationFunctionType
NPBF = ml_dtypes.bfloat16

D = 128
SP = 1280
FFN = 256
NB = 3
DH = 32
KV_IDX = ((1, 2), (0, 2), (0, 1))
NCORES = 8
BLK = 1024
MMN = 512
EPS = 1e-5
ISQ = float(1.0 / np.sqrt(DH))

# tuning flags
IDENT_FOLD = True      # residual adds via identity matmuls on PE

# filled by build_program: [(phase_label, [instruction names]), ...]
PHASE_MARKS = []


def _patch_tile_drain():
    """walrus here rejects >4 sem waits on one instruction; Tile's tail
    drain carries one wait per logical proc.  Re-emit them as standalone
    wait_ge instructions ahead of the drain."""
    TC = TileContext
    if getattr(TC, "_drain_patched", False):
        return

    def patched(self, tick_clock, wait_clock):
        nop_inst = self.nc.sync.nop()
        wait_clock.add_sem_waits(
            nop_inst.ins, ScopedClock({None: tick_clock.global_clock})
        )
        d = nop_inst.ins
        si = d.sync_info
        waits = list(si.on_wait) if si is not None else []
        if len(waits) > 4:
            si.on_wait = []
            d.sync_info = si
            name2sem = {s.name: s for s in self.sems.allocated().values()}
            for w in waits:
                sem = name2sem.get(w.ant_name)
                if sem is None:
                    raise RuntimeError(f"drain patch: unknown sem {w.ant_name}")
                self.nc.sync.wait_ge(sem, w.wait_value)
        self.nc.sync.drain()
        self.nc.all_engine_barrier()
        popped = self.nc._tile_sem_poison_stack.pop()
        assert popped is self._sem_poison
        self.nc.clear_and_free_semaphores(list(self.sems.allocated().values()))
        self.nc.all_engine_barrier()

    TC._drain_and_barrier = patched
    TC._drain_patched = True


def _fix_wait_overflow(nc):
    """walrus enforces per-opcode caps on sync-wait commands attached to
    one instruction (DmaTransposeAnt: 1, others: ~4).  Move the excess
    onto same-engine NOPs inserted immediately before the instruction."""
    LIMITS = {}
    DEFAULT_LIM = 1
    for fn in nc.m.functions:
        for bb in fn.blocks:
            insts = list(bb.instructions)
            out = []
            changed = False
            for inst in insts:
                si = getattr(inst, "sync_info", None)
                w = list(si.on_wait) if si is not None and si.on_wait else []
                lim = LIMITS.get(type(inst).__name__, DEFAULT_LIM)
                if len(w) > lim:
                    excess = w[lim:]
                    keep = w[:lim]
                    eng = nc.engines[inst.engine]
                    nops = []
                    for i in range(0, len(excess), 1):
                        chunk = excess[i:i + 1]
                        nop_bi = eng.nop()
                        nop_inst = nop_bi.ins
                        cb = nc.cur_bb.bb
                        cb.instructions = [x for x in cb.instructions
                                           if x.name != nop_inst.name]
                        import bass_rust
                        nop_inst.sync_info = bass_rust.SyncInfo(
                            on_wait=chunk, on_update=[])
                        nops.append(nop_inst)
                    si.on_wait = keep
                    inst.sync_info = si
                    out.extend(nops)
                    changed = True
                out.append(inst)
            if changed:
                bb.instructions = out


def prep_weights(inp):
    """Host-side prep of all weights into SBUF layouts. bf16 for matmul
    operands, fp32 for per-partition bias vectors."""
    f64 = np.float64
    C = np.eye(D, dtype=f64) - 1.0 / D

    def bf(a):
        return np.ascontiguousarray(a.astype(np.float32)).astype(NPBF)

    def f32(a):
        return np.ascontiguousarray(a, dtype=np.float32)

    w = {}
    wsp = C @ inp["proj_w_spatial"].astype(f64)            # [128,1280]
    w["wspT"] = bf(np.transpose(wsp.reshape(D, 10, D), (2, 1, 0)).reshape(D, 10 * D))
    wgf = np.stack([C @ inp["proj_w_gf"][i].astype(f64) for i in range(2)])
    w["wgfT"] = bf(np.transpose(wgf, (2, 0, 1)).reshape(D, 2 * D))
    w["bc"] = f32(C @ inp["proj_b"].astype(f64).T)         # [128,3]
    w["emb"] = f32(inp["mod_emb"].T)

    ipw = inp["in_proj_w"].astype(f64)                     # [3, 384, 128]
    wq, wk, wv = ipw[:, :D], ipw[:, D:2 * D], ipw[:, 2 * D:]
    w["wqT"] = bf(np.transpose(wq, (2, 0, 1)).reshape(D, NB * D))
    w["wkT"] = bf(np.transpose(wk, (2, 0, 1)).reshape(D, NB * D))
    w["wvT"] = bf(np.transpose(wv, (2, 0, 1)).reshape(D, NB * D))
    ow = np.stack([C @ inp["out_proj_w"][n].astype(f64) for n in range(NB)])
    # 0.5x fold: attention prob a = (1+tanh(d/(2 sqrt(dh))))/2, the 1/2 is
    # folded here so tp = (tanh+1)*dv feeds Wo directly
    w["owT"] = bf(0.5 * np.transpose(ow, (2, 0, 1)).reshape(D, NB * D))
    # Wov[n] = (C @ Wo[n]) @ Wv[n] : folds the v1 path into one matmul
    wov = np.stack([ow[n] @ wv[n] for n in range(NB)])
    w["wovT"] = bf(np.transpose(wov, (2, 0, 1)).reshape(D, NB * D))
    ob2 = np.stack([
        C @ inp["out_proj_b"][n].astype(f64)
        - inp["mod_emb"][n].astype(f64).mean()
        for n in range(NB)])
    w["ob2"] = f32(ob2.T)

    w1 = inp["ffn_w1"].astype(f64)                         # [3, 256, 128]
    w["w1T"] = bf(np.transpose(w1, (2, 0, 1)).reshape(D, NB * FFN))
    w["b1"] = f32(inp["ffn_b1"].reshape(NB * 2, D).T)      # [128, 6]
    w2 = np.stack([C @ inp["ffn_w2"][n].astype(f64) for n in range(NB)])
    w2c = w2.reshape(NB, D, 2, D)                          # [n, j, c, p]
    w["w2T"] = bf(np.transpose(w2c, (3, 0, 2, 1)).reshape(D, NB * 2 * D))
    b2c = np.stack([C @ inp["ffn_b2"][n].astype(f64) for n in range(NB)])
    w["b2c"] = f32(b2c.T)

    gw = inp["gate_w"].astype(f64).reshape(NB, NB, D)      # [j, n, p]
    w["gwT"] = bf(np.transpose(gw, (2, 1, 0)).reshape(D, NB * NB))
    w["gateb"] = f32(inp["gate_b"].reshape(NB, 1))

    w["onesT"] = bf(np.full((D, D), 1.0 / D))
    svsel = np.zeros((D, NB * NB), dtype=np.float32)
    for n in range(NB):
        svsel[:, NB * n + n] = 1.0 / D
    w["svsel"] = bf(svsel)
    hs = np.zeros((D, D), dtype=np.float32)
    for h in range(4):
        hs[h * DH:(h + 1) * DH, h * DH:(h + 1) * DH] = 1.0
    w["hsel"] = bf(hs)
    w["ones3"] = bf(np.ones((NB, D)))
    esel = np.zeros((NB, NB * D), dtype=np.float32)
    for n in range(NB):
        esel[n, n * D:(n + 1) * D] = 1.0
    w["esel"] = bf(esel)
    esl2 = np.zeros((32 + NB, NB * D), dtype=np.float32)
    esl3 = np.zeros((64 + NB, NB * D), dtype=np.float32)
    for n in range(NB):
        esl2[32 + n, n * D:(n + 1) * D] = 1.0
        esl3[64 + n, n * D:(n + 1) * D] = 1.0
    w["esl2"] = bf(esl2)
    w["esl3"] = bf(esl3)
    w["ident"] = bf(np.eye(D))
    w["epsv"] = np.full((D, 1), EPS, dtype=np.float32)
    w["zerov"] = np.zeros((D, 1), dtype=np.float32)

    assert np.allclose(inp["proj_ln_g"], 1) and np.allclose(inp["proj_ln_b"], 0)
    assert np.allclose(inp["attn_ln_g"], 1) and np.allclose(inp["attn_ln_b"], 0)
    assert np.allclose(inp["ffn_ln_g"], 1) and np.allclose(inp["ffn_ln_b"], 0)
    assert np.allclose(inp["in_proj_b"], 0)
    return w


WEIGHT_SPECS = {
    "wspT": ((D, 10 * D), BF16), "wgfT": ((D, 2 * D), BF16),
    "bc": ((D, NB), F32), "emb": ((D, NB), F32),
    "wqT": ((D, NB * D), BF16), "wkT": ((D, NB * D), BF16),
    "wvT": ((D, NB * D), BF16), "owT": ((D, NB * D), BF16),
    "wovT": ((D, NB * D), BF16),
    "ob2": ((D, NB), F32),
    "w1T": ((D, NB * FFN), BF16), "b1": ((D, NB * 2), F32),
    "w2T": ((D, NB * 2 * D), BF16), "b2c": ((D, NB), F32),
    "gwT": ((D, NB * NB), BF16), "gateb": ((NB, 1), F32),
    "onesT": ((D, D), BF16), "hsel": ((D, D), BF16),
    "svsel": ((D, NB * NB), BF16),
    "ones3": ((NB, D), BF16), "esel": ((NB, NB * D), BF16),
    "esl2": ((32 + NB, NB * D), BF16), "esl3": ((64 + NB, NB * D), BF16),
    "ident": ((D, D), BF16),
    "epsv": ((D, 1), F32), "zerov": ((D, 1), F32),
}


def build_program(Bc, repeat=1):
    nc = bass.Bass()
    # pre-transposed feature-major inputs in HBM
    xsp = nc.dram_tensor("xspT", [D, 10, Bc], BF16, kind="ExternalInput")
    xg = nc.dram_tensor("xgT", [D, Bc], BF16, kind="ExternalInput")
    xf = nc.dram_tensor("xfT", [D, Bc], BF16, kind="ExternalInput")
    wd = {k: nc.dram_tensor(k, list(s[0]), s[1], kind="ExternalInput")
          for k, s in WEIGHT_SPECS.items()}
    out = nc.dram_tensor("outT", [D, Bc], BF16, kind="ExternalOutput")

    nblk = Bc // BLK
    assert Bc % BLK == 0

    with TileContext(nc) as tc, nc.allow_low_precision(reason="bf16 kernel"):
        with (
            tc.tile_pool(name="wp", bufs=1) as wp,
            tc.tile_pool(name="xin", bufs=2) as xin,
            tc.tile_pool(name="work", bufs=2) as wk_,
            tc.tile_pool(name="ps", bufs=8, space="PSUM") as psp,
        ):
            W = {}
            for k, s in WEIGHT_SPECS.items():
                W[k] = wp.tile(list(s[0]), s[1], tag=k, name=k)
                nc.gpsimd.dma_start(W[k][:], wd[k][:])
            ident = W["ident"]
            # shared LN-stats scratch: the 3 pipeline stages land their
            # Ln outputs at partition offsets 0/32/64 of one tile, so a
            # single Exp finishes all three rsqrt computations per tick
            lnvp = wp.tile([64 + NB, BLK], F32, tag="lnvp", name="lnvp")
            nc.vector.memset(lnvp[:], 0.0)

            def mm(out_ap, lhsT, rhs, start=True, stop=True):
                for h in range(BLK // MMN):
                    nc.tensor.matmul(out_ap[:, h * MMN:(h + 1) * MMN], lhsT,
                                     rhs[:, h * MMN:(h + 1) * MMN],
                                     start=start, stop=stop)

            def phase0(b):
                r0 = (b % nblk) * BLK
                st = {}
                for half, nmh in ((0, "xspA"), (1, "xspB")):
                    xt = xin.tile([D, 5 * BLK], BF16, tag="xspT", bufs=3,
                                  name=nmh)
                    nc.sync.dma_start(
                        xt[:].rearrange("p (c n) -> p c n", c=5),
                        xsp[:, 5 * half:5 * (half + 1), r0:r0 + BLK])
                    st[nmh] = xt
                st["xgT"] = xin.tile([D, BLK], BF16, tag="xgT", name="xgT")
                nc.sync.dma_start(st["xgT"][:], xg[:, r0:r0 + BLK])
                st["xfT"] = xin.tile([D, BLK], BF16, tag="xfT", name="xfT")
                nc.sync.dma_start(st["xfT"][:], xf[:, r0:r0 + BLK])
                return st

            NH = BLK // MMN   # psum halves per logical [D, BLK] tile

            def hmm(name, pairs, parts=D, poff=0):
                """Accumulating matmul into NH independent psum half tiles.
                pairs: [(lhsT_ap, full-width SBUF rhs AP)] accumulated.
                Emits half 0 fully, then half 1 (so half-0 consumers can
                start while half 1 accumulates).  Returns list of halves."""
                halves = []
                for h in range(NH):
                    pt = psp.tile([D, MMN], F32, tag="ps", name=f"{name}_{h}")
                    ap = pt[poff:poff + parts, :] if parts != D else pt[:]
                    for i, (lhsT, rhs) in enumerate(pairs):
                        nc.tensor.matmul(ap, lhsT,
                                         rhs[:, h * MMN:(h + 1) * MMN],
                                         start=(i == 0),
                                         stop=(i == len(pairs) - 1))
                    halves.append((pt, ap))
                return halves

            def for_halves(halves, sbuf_op):
                """sbuf_op(h, lo, hi, psum_ap) for each half."""
                for h, (pt, ap) in enumerate(halves):
                    sbuf_op(h, h * MMN, (h + 1) * MMN, ap)

            def stage_ln(sv_halves, off):
                """per-stage Ln of mean-squares into lnvp rows
                [off, off+NB); a single Exp per tick (in pS) finishes
                rsqrt for all three stages at once."""
                for_halves(sv_halves, lambda h, lo, hi, ap:
                           nc.scalar.activation(
                               lnvp[off:off + NB, lo:hi], ap, AF.Ln,
                               bias=W["epsv"][off:off + NB, 0:1]))

            def bcast_apply(rbca, stage, n, src_sb, out_sb, name):
                """out = src * broadcast(rbca row of stage/branch), per
                half.  Stage s reads rbca rows [0, 32*s+NB) against a
                zero-padded selector so only row 32*s+n contributes."""
                selw = (W["esel"], W["esl2"], W["esl3"])[stage]
                rows = 32 * stage + NB
                sel = selw[:rows, n * D:(n + 1) * D]
                for h in range(NH):
                    lo, hi = h * MMN, (h + 1) * MMN
                    rbb = psp.tile([D, MMN], F32, tag="ps",
                                   name=f"{name}_{h}")
                    nc.tensor.matmul(rbb[:], sel, rbca[:rows, lo:hi],
                                     start=True, stop=True)
                    nc.vector.tensor_tensor(out_sb[:, lo:hi], src_sb[:, lo:hi],
                                            rbb[:], AL.mult)

            def p1a(st):
                """projections + drains + squares + LN1 stats/rsqrt"""
                zh = []
                zh.append(hmm("zsp", [
                    (W["wspT"][:, c * D:(c + 1) * D],
                     st["xspA" if c < 5 else "xspB"][:, (c % 5) * BLK:
                                                     (c % 5 + 1) * BLK])
                    for c in range(10)]))
                zh.append(hmm("zg", [(W["wgfT"][:, 0:D], st["xgT"][:])]))
                zh.append(hmm("zf", [(W["wgfT"][:, D:2 * D], st["xfT"][:])]))
                zsb, sq = [], []
                for n in range(NB):
                    z_sb = wk_.tile([D, BLK], BF16, tag=f"zsb{n}", bufs=2)
                    for_halves(zh[n], lambda h, lo, hi, ap:
                               nc.scalar.activation(z_sb[:, lo:hi], ap,
                                                    AF.Identity,
                                                    bias=W["bc"][:, n:n + 1]))
                    zsb.append(z_sb)
                    s_ = wk_.tile([D, BLK], BF16, tag="sq1", bufs=1)
                    nc.vector.tensor_tensor(s_[:], z_sb[:], z_sb[:], AL.mult)
                    sq.append(s_)
                sv = hmm("sv1", [(W["svsel"][:, NB * n:NB * (n + 1)],
                                  sq[n][:]) for n in range(NB)], parts=NB)
                stage_ln(sv, 0)
                st["zsb"] = zsb

            def p1b(st, rbca):
                """LN1 broadcast + apply + emb + dP"""
                zsb = st.pop("zsb")
                P = []
                for n in range(NB):
                    p_ = wk_.tile([D, BLK], BF16, tag=f"P{n}")
                    bcast_apply(rbca, 0, n, zsb[n], p_, f"rbb1_{n}")
                    nc.vector.tensor_scalar_add(p_[:], p_[:],
                                                W["emb"][:, n:n + 1])
                    P.append(p_)
                st["P"] = P
                dP = []
                for n in range(NB):
                    s0, s1 = KV_IDX[n]
                    dp = wk_.tile([D, BLK], BF16, tag=f"dP{n}", bufs=2)
                    nc.vector.tensor_tensor(dp[:], P[s0][:], P[s1][:],
                                            AL.subtract)
                    dP.append(dp)
                st["dP"] = dP

            def p2a(st):
                """q/dk matmuls, q drain, score product"""
                P, dP = st["P"], st["dP"]
                t0 = {}
                for n in range(NB):
                    qh = hmm(f"q{n}", [(W["wqT"][:, n * D:(n + 1) * D],
                                        P[n][:])])
                    dkh = hmm(f"dk{n}", [(W["wkT"][:, n * D:(n + 1) * D],
                                          dP[n][:])])
                    q_sb = wk_.tile([D, BLK], BF16, tag="qsb", bufs=2,
                                    name=f"qsb{n}")
                    for_halves(qh, lambda h, lo, hi, ap:
                               nc.scalar.activation(q_sb[:, lo:hi], ap,
                                                    AF.Copy))
                    t0[n] = wk_.tile([D, BLK], BF16, tag="t0", bufs=2,
                                     name=f"t0{n}")
                    for_halves(dkh, lambda h, lo, hi, ap:
                               nc.vector.tensor_tensor(t0[n][:, lo:hi],
                                                       q_sb[:, lo:hi], ap,
                                                       AL.mult))
                st["t0"] = t0

            def p2b(st):
                """dv/score-bcast matmuls, tanh, fused (tanh+1)*dv"""
                dP = st["dP"]
                t0 = st.pop("t0")
                tp = {}
                for n in range(NB):
                    dvh = hmm(f"dv{n}", [(W["wvT"][:, n * D:(n + 1) * D],
                                          dP[n][:])])
                    dh = hmm(f"d{n}", [(W["hsel"][:], t0[n][:])])
                    th = wk_.tile([D, BLK], BF16, tag="th", bufs=2,
                                  name=f"th{n}")
                    for_halves(dh, lambda h, lo, hi, ap:
                               nc.scalar.activation(th[:, lo:hi], ap, AF.Tanh,
                                                    bias=W["zerov"][:, 0:1],
                                                    scale=0.5 * ISQ))
                    tp[n] = wk_.tile([D, BLK], BF16, tag="tp", bufs=2,
                                     name=f"tp{n}")
                    for_halves(dvh, lambda h, lo, hi, ap:
                               nc.vector.scalar_tensor_tensor(
                                   tp[n][:, lo:hi], th[:, lo:hi], 1.0, ap,
                                   AL.add, AL.mult))
                st["tp"] = tp

            def p2c(st):
                """attention out + residual (PE-folded) + LN2 stats/rsqrt"""
                P = st["P"]
                tp = st.pop("tp")
                us, sqs = [], []
                for n in range(NB):
                    s0, s1 = KV_IDX[n]
                    oh = hmm(f"o{n}",
                             [(W["owT"][:, n * D:(n + 1) * D], tp[n][:]),
                              (W["wovT"][:, n * D:(n + 1) * D], P[s1][:]),
                              (ident[:], P[n][:])])
                    u = wk_.tile([D, BLK], BF16, tag=f"u{n}", bufs=2)
                    for_halves(oh, lambda h, lo, hi, ap:
                               nc.scalar.activation(u[:, lo:hi], ap,
                                                    AF.Identity,
                                                    bias=W["ob2"][:, n:n + 1]))
                    us.append(u)
                    s_ = wk_.tile([D, BLK], BF16, tag="sq2", bufs=1)
                    nc.vector.tensor_tensor(s_[:], u[:], u[:], AL.mult)
                    sqs.append(s_)
                sv = hmm("sv2", [(W["svsel"][:, NB * n:NB * (n + 1)],
                                  sqs[n][:]) for n in range(NB)],
                         parts=NB, poff=32)
                stage_ln(sv, 32)
                st["us"] = us

            def p2d(st, rbca):
                """LN2 broadcast + apply"""
                us = st.pop("us")
                x1 = []
                for n in range(NB):
                    # x1 lives 3 ticks: made here, read by p3a and p3b1
                    x1n = wk_.tile([D, BLK], BF16, tag=f"x1{n}", bufs=3)
                    bcast_apply(rbca, 1, n, us[n], x1n, f"rbb2_{n}")
                    x1.append(x1n)
                st["x1"] = x1

            def phase3a(st):
                """FFN first half: W1 matmuls + gelu cluster."""
                x1 = st["x1"]
                hs_all = []
                for n in range(NB):
                    h_sb = []
                    for c in range(2):
                        hh = hmm(f"h{n}_{c}",
                                 [(W["w1T"][:, n * FFN + c * D:
                                            n * FFN + (c + 1) * D],
                                   x1[n][:])])
                        hs_ = wk_.tile([D, BLK], BF16, tag=f"hsb{n}_{c}",
                                       bufs=1)
                        for_halves(hh, lambda h, lo, hi, ap:
                                   nc.scalar.activation(
                                       hs_[:, lo:hi], ap, AF.Gelu,
                                       bias=W["b1"][:, 2 * n + c:
                                                    2 * n + c + 1]))
                        h_sb.append(hs_)
                    hs_all.append(h_sb)
                st["hs"] = hs_all

            def p3b1(st):
                """FFN second half + residual + LN3 stats/rsqrt"""
                x1 = st["x1"]
                x2ps, sqs = [], []
                for n in range(NB):
                    h_sb = st["hs"][n]
                    fh = hmm(f"f{n}",
                             [(W["w2T"][:, (2 * n) * D:(2 * n + 1) * D],
                               h_sb[0][:]),
                              (W["w2T"][:, (2 * n + 1) * D:(2 * n + 2) * D],
                               h_sb[1][:]),
                              (ident[:], x1[n][:])])
                    x2p = wk_.tile([D, BLK], BF16, tag=f"x2p{n}", bufs=2)
                    for_halves(fh, lambda h, lo, hi, ap:
                               nc.scalar.activation(x2p[:, lo:hi], ap,
                                                    AF.Identity,
                                                    bias=W["b2c"][:, n:n + 1]))
                    x2ps.append(x2p)
                    s_ = wk_.tile([D, BLK], BF16, tag="sq3", bufs=1)
                    nc.vector.tensor_tensor(s_[:], x2p[:], x2p[:], AL.mult)
                    sqs.append(s_)
                sv = hmm("sv3", [(W["svsel"][:, NB * n:NB * (n + 1)],
                                  sqs[n][:]) for n in range(NB)],
                         parts=NB, poff=64)
                stage_ln(sv, 64)
                st["x2ps"] = x2ps

            def p3b2(st, rbca):
                """LN3 broadcast + apply"""
                x2ps = st.pop("x2ps")
                st.pop("hs")
                x2 = []
                for n in range(NB):
                    x2n = wk_.tile([D, BLK], BF16, tag=f"x2{n}")
                    bcast_apply(rbca, 2, n, x2ps[n], x2n, f"rbb3_{n}")
                    x2.append(x2n)
                st["x2"] = x2

            def phase4(st, b):
                r0 = (b % nblk) * BLK
                x2 = st["x2"]
                gh = hmm("g", [(W["gwT"][:, n * NB:(n + 1) * NB], x2[n][:])
                               for n in range(NB)], parts=NB)
                e_sb = wk_.tile([NB, BLK], BF16, tag="esb", bufs=1)
                for_halves(gh, lambda h, lo, hi, ap:
                           nc.scalar.activation(e_sb[:, lo:hi], ap, AF.Exp,
                                                bias=W["gateb"][:NB, 0:1]))
                zbh = hmm("zb", [(W["ones3"][:NB, :], e_sb[:])])
                rz = wk_.tile([D, BLK], BF16, tag="rz", bufs=1)
                for_halves(zbh, lambda h, lo, hi, ap:
                           nc.vector.reciprocal(rz[:, lo:hi], ap))
                mns = []
                for n in range(NB):
                    ebh = hmm(f"eb{n}", [(W["esel"][:NB, n * D:(n + 1) * D],
                                          e_sb[:])])
                    mn = wk_.tile([D, BLK], BF16, tag=f"mn{n}", bufs=1)
                    for_halves(ebh, lambda h, lo, hi, ap:
                               nc.vector.tensor_tensor(mn[:, lo:hi],
                                                       x2[n][:, lo:hi], ap,
                                                       AL.mult))
                    mns.append(mn)
                acc = wk_.tile([D, BLK], BF16, tag="macc", bufs=1)
                nc.vector.tensor_tensor(acc[:], mns[0][:], mns[1][:], AL.add)
                acc2 = wk_.tile([D, BLK], BF16, tag="macc2", bufs=1)
                nc.vector.tensor_tensor(acc2[:], acc[:], mns[2][:], AL.add)
                fused = wk_.tile([D, BLK], BF16, tag="fused", bufs=1)
                nc.vector.tensor_tensor(fused[:], acc2[:], rz[:], AL.mult)
                nc.gpsimd.dma_start(out[:, r0:r0 + BLK], fused[:])

            # sub-phase interleaved emission, 6 blocks in flight; psum is
            # 8 independent [D, MMN] half-tile slots.  ACT table sets:
            # gelu+tanh (gelu_and_others) at the tick head, all Ln/Exp
            # users after -> 2 table switches per tick.
            total = nblk * repeat
            bstate = {}
            rbca_prev = rbca_cur = None
            marks = PHASE_MARKS
            marks.clear()

            def _run(label, fn, *a):
                i0 = len(nc.cur_bb.bb.instructions)
                fn(*a)
                i1 = len(nc.cur_bb.bb.instructions)
                marks.append((label, [x.name for x in
                                      nc.cur_bb.bb.instructions[i0:i1]]))

            for t in range(total + 8):
                if t < total:
                    bstate[t] = None
                    _run("p0", lambda tt=t: bstate.__setitem__(tt, phase0(tt)))
                if 0 <= t - 5 < total:
                    _run("p3a", phase3a, bstate[t - 5])
                if 0 <= t - 3 < total:
                    _run("p2a", p2a, bstate[t - 3])
                    _run("p2b", p2b, bstate[t - 3])
                if 0 <= t - 1 < total:
                    _run("p1a", p1a, bstate[t - 1])
                if 0 <= t - 3 < total:
                    _run("p2c", p2c, bstate[t - 3])
                if 0 <= t - 7 < total:
                    _run("p4", phase4, bstate[t - 7], t - 7)
                    bstate.pop(t - 7)
                if 0 <= t - 5 < total:
                    _run("p3b1", p3b1, bstate[t - 5])
                # apply sections ordered by next-tick consumer priority:
                # p2d feeds next tick's head (p3a), p1b the 2nd section
                # (p2a), p3b2 the tail (p4)
                if 0 <= t - 4 < total:
                    _run("p2d", p2d, bstate[t - 4], rbca_prev)
                if 0 <= t - 2 < total:
                    _run("p1b", p1b, bstate[t - 2], rbca_prev)
                if 0 <= t - 6 < total:
                    _run("p3b2", p3b2, bstate[t - 6], rbca_prev)
                if any(0 <= t - k < total for k in (1, 3, 5)):
                    # one Exp finishes rsqrt for all three LN stages
                    rbca_cur = wk_.tile([64 + NB, BLK], BF16, tag="rbca",
                                        bufs=2, name=f"rbca{t % 2}")
                    i0 = len(nc.cur_bb.bb.instructions)
                    nc.scalar.activation(rbca_cur[:], lnvp[:], AF.Exp,
                                         scale=-0.5,
                                         bias=W["zerov"][0:64 + NB, 0:1])
                    marks.append(("pS", [x.name for x in
                                         nc.cur_bb.bb.instructions[i0:]]))
                rbca_prev = rbca_cur
    _fix_wait_overflow(nc)
    return nc


def prep_x(inputs, Bc=None):
    """Host-side: cast to bf16 and pre-transpose into feature-major HBM
    layouts."""
    xsp = np.ascontiguousarray(inputs["x_spatial"]).astype(NPBF)
    B = xsp.shape[0]
    xspT = np.ascontiguousarray(xsp.reshape(B, 10, D).transpose(2, 1, 0))
    xgT = np.ascontiguousarray(inputs["x_gradient"].T.astype(NPBF))
    xfT = np.ascontiguousarray(inputs["x_frequency"].T.astype(NPBF))
    return {"xspT": xspT, "xgT": xgT, "xfT": xfT}


def kernel(**inputs):
    _patch_tile_drain()
    B = inputs["x_spatial"].shape[0]
    Bc = B // NCORES
    w = prep_weights(inputs)
    xb = prep_x(inputs)
    nc = build_program(Bc)
    in_maps = []
    for c in range(NCORES):
        m = dict(w)
        m["xspT"] = np.ascontiguousarray(xb["xspT"][:, :, c * Bc:(c + 1) * Bc])
        m["xgT"] = np.ascontiguousarray(xb["xgT"][:, c * Bc:(c + 1) * Bc])
        m["xfT"] = np.ascontiguousarray(xb["xfT"][:, c * Bc:(c + 1) * Bc])
        in_maps.append(m)
    res = run_bass_kernel_spmd(nc, in_maps, list(range(NCORES)))
    outs = [res.results[c]["outT"] for c in range(NCORES)]
    full = np.concatenate([o.T for o in outs], axis=0)
    return np.ascontiguousarray(full.astype(np.float32))


# revision 38
# speedup vs baseline: 1.2984x; 1.2984x over previous
"""Trainium2 Bass kernel for nn_CMAF (cross-modal attention fusion block).

Layout: feature-major activations on-chip — every tile is
[128 features (partitions) x 1024 samples (free)], so all matmuls are
weight-stationary bf16 with the batch as the moving free dimension.
Inputs are pre-transposed host-side into feature-major HBM layouts, so
device DMA is fully contiguous (no DMA-transpose).

Engine-balance design (ACT/DVE were the baseline bottleneck):
 - LN stats (sum of squares) for all 3 branches land in ONE [3,1024]
   PSUM tile; Ln+Exp (rsqrt) run once per LN stage on that compact tile
   instead of per-branch full tiles; per-branch ones-matmuls broadcast
   the result back to 128 partitions (PE pump is cheap).
 - Residual adds (u = o + P, x2p = f + x1) are folded into the PE as
   identity-matrix accumulation matmuls, killing 1x-rate STT DVE ops.
 - Wo@v1 is folded host-side into Wov = (C Wo) Wv and accumulated into
   the same PSUM as Wo@tp, killing the tpv add.
 - The 2-way attention softmax collapses to division by (1+exp(-d/sqrt(dh)))
   done as a single DVE tensor_tensor divide straight from PSUM.
 - Gelu ACT ops are clustered at alternating head/tail of the pipeline
   tick so the ACT table set (gelu vs natural_log_exp) switches once per
   block on average instead of twice.

Data parallel over 8 NeuronCores: 8192 samples each.
"""

import numpy as np
import ml_dtypes

import concourse.bass as bass
import concourse.mybir as mybir
from concourse.tile import TileContext
from concourse.vector_clock import ScopedClock
from concourse.bass_utils import run_bass_kernel_spmd

F32 = mybir.dt.float32
BF16 = mybir.dt.bfloat16
AL = mybir.AluOpType
AF = mybir.ActivationFunctionType
NPBF = ml_dtypes.bfloat16

D = 128
SP = 1280
FFN = 256
NB = 3
DH = 32
KV_IDX = ((1, 2), (0, 2), (0, 1))
NCORES = 8
BLK = 1024
MMN = 512
EPS = 1e-5
ISQ = float(1.0 / np.sqrt(DH))

# tuning flags
IDENT_FOLD = True      # residual adds via identity matmuls on PE

# filled by build_program: [(phase_label, [instruction names]), ...]
PHASE_MARKS = []


def _patch_tile_drain():
    """walrus here rejects >4 sem waits on one instruction; Tile's tail
    drain carries one wait per logical proc.  Re-emit them as standalone
    wait_ge instructions ahead of the drain."""
    TC = TileContext
    if getattr(TC, "_drain_patched", False):
        return

    def patched(self, tick_clock, wait_clock):
        nop_inst = self.nc.sync.nop()
        wait_clock.add_sem_waits(
            nop_inst.ins, ScopedClock({None: tick_clock.global_clock})
        )
        d = nop_inst.ins
        si = d.sync_info
        waits = list(si.on_wait) if si is not None else []
        if len(waits) > 4:
            si.on_wait = []
            d.sync_info = si
            name2sem = {s.name: s for s in self.sems.allocated().values()}
            for w in waits:
                sem = name2sem.get(w.ant_name)
                if sem is None:
                    raise RuntimeError(f"drain patch: unknown sem {w.ant_name}")
                self.nc.sync.wait_ge(sem, w.wait_value)
        self.nc.sync.drain()
        self.nc.all_engine_barrier()
        popped = self.nc._tile_sem_poison_stack.pop()
        assert popped is self._sem_poison
        self.nc.clear_and_free_semaphores(list(self.sems.allocated().values()))
        self.nc.all_engine_barrier()

    TC._drain_and_barrier = patched
    TC._drain_patched = True


def _fix_wait_overflow(nc):
    """walrus enforces per-opcode caps on sync-wait commands attached to
    one instruction (DmaTransposeAnt: 1, others: ~4).  Move the excess
    onto same-engine NOPs inserted immediately before the instruction."""
    LIMITS = {}
    DEFAULT_LIM = 1
    for fn in nc.m.functions:
        for bb in fn.blocks:
            insts = list(bb.instructions)
            out = []
            changed = False
            for inst in insts:
                si = getattr(inst, "sync_info", None)
                w = list(si.on_wait) if si is not None and si.on_wait else []
                lim = LIMITS.get(type(inst).__name__, DEFAULT_LIM)
                if len(w) > lim:
                    excess = w[lim:]
                    keep = w[:lim]
                    eng = nc.engines[inst.engine]
                    nops = []
                    for i in range(0, len(excess), 1):
                        chunk = excess[i:i + 1]
                        nop_bi = eng.nop()
                        nop_inst = nop_bi.ins
                        cb = nc.cur_bb.bb
                        cb.instructions = [x for x in cb.instructions
                                           if x.name != nop_inst.name]
                        import bass_rust
                        nop_inst.sync_info = bass_rust.SyncInfo(
                            on_wait=chunk, on_update=[])
                        nops.append(nop_inst)
                    si.on_wait = keep
                    inst.sync_info = si
                    out.extend(nops)
                    changed = True
                out.append(inst)
            if changed:
                bb.instructions = out


def prep_weights(inp):
    """Host-side prep of all weights into SBUF layouts. bf16 for matmul
    operands, fp32 for per-partition bias vectors."""
    f64 = np.float64
    C = np.eye(D, dtype=f64) - 1.0 / D

    def bf(a):
        return np.ascontiguousarray(a.astype(np.float32)).astype(NPBF)

    def f32(a):
        return np.ascontiguousarray(a, dtype=np.float32)

    w = {}
    wsp = C @ inp["proj_w_spatial"].astype(f64)            # [128,1280]
    w["wspT"] = bf(np.transpose(wsp.reshape(D, 10, D), (2, 1, 0)).reshape(D, 10 * D))
    wgf = np.stack([C @ inp["proj_w_gf"][i].astype(f64) for i in range(2)])
    w["wgfT"] = bf(np.transpose(wgf, (2, 0, 1)).reshape(D, 2 * D))
    w["bc"] = f32(C @ inp["proj_b"].astype(f64).T)         # [128,3]
    w["emb"] = f32(inp["mod_emb"].T)

    ipw = inp["in_proj_w"].astype(f64)                     # [3, 384, 128]
    wq, wk, wv = ipw[:, :D], ipw[:, D:2 * D], ipw[:, 2 * D:]
    w["wqT"] = bf(np.transpose(wq, (2, 0, 1)).reshape(D, NB * D))
    w["wkT"] = bf(np.transpose(wk, (2, 0, 1)).reshape(D, NB * D))
    w["wvT"] = bf(np.transpose(wv, (2, 0, 1)).reshape(D, NB * D))
    ow = np.stack([C @ inp["out_proj_w"][n].astype(f64) for n in range(NB)])
    # 0.5x fold: attention prob a = (1+tanh(d/(2 sqrt(dh))))/2, the 1/2 is
    # folded here so tp = (tanh+1)*dv feeds Wo directly
    w["owT"] = bf(0.5 * np.transpose(ow, (2, 0, 1)).reshape(D, NB * D))
    # Wov[n] = (C @ Wo[n]) @ Wv[n] : folds the v1 path into one matmul
    wov = np.stack([ow[n] @ wv[n] for n in range(NB)])
    w["wovT"] = bf(np.transpose(wov, (2, 0, 1)).reshape(D, NB * D))
    ob2 = np.stack([
        C @ inp["out_proj_b"][n].astype(f64)
        - inp["mod_emb"][n].astype(f64).mean()
        for n in range(NB)])
    w["ob2"] = f32(ob2.T)

    w1 = inp["ffn_w1"].astype(f64)                         # [3, 256, 128]
    w["w1T"] = bf(np.transpose(w1, (2, 0, 1)).reshape(D, NB * FFN))
    w["b1"] = f32(inp["ffn_b1"].reshape(NB * 2, D).T)      # [128, 6]
    w2 = np.stack([C @ inp["ffn_w2"][n].astype(f64) for n in range(NB)])
    w2c = w2.reshape(NB, D, 2, D)                          # [n, j, c, p]
    w["w2T"] = bf(np.transpose(w2c, (3, 0, 2, 1)).reshape(D, NB * 2 * D))
    b2c = np.stack([C @ inp["ffn_b2"][n].astype(f64) for n in range(NB)])
    w["b2c"] = f32(b2c.T)

    gw = inp["gate_w"].astype(f64).reshape(NB, NB, D)      # [j, n, p]
    w["gwT"] = bf(np.transpose(gw, (2, 1, 0)).reshape(D, NB * NB))
    w["gateb"] = f32(inp["gate_b"].reshape(NB, 1))

    w["onesT"] = bf(np.full((D, D), 1.0 / D))
    svsel = np.zeros((D, NB * NB), dtype=np.float32)
    for n in range(NB):
        svsel[:, NB * n + n] = 1.0 / D
    w["svsel"] = bf(svsel)
    hs = np.zeros((D, D), dtype=np.float32)
    for h in range(4):
        hs[h * DH:(h + 1) * DH, h * DH:(h + 1) * DH] = 1.0
    w["hsel"] = bf(hs)
    w["ones3"] = bf(np.ones((NB, D)))
    esel = np.zeros((NB, NB * D), dtype=np.float32)
    for n in range(NB):
        esel[n, n * D:(n + 1) * D] = 1.0
    w["esel"] = bf(esel)
    esl2 = np.zeros((32 + NB, NB * D), dtype=np.float32)
    esl3 = np.zeros((64 + NB, NB * D), dtype=np.float32)
    for n in range(NB):
        esl2[32 + n, n * D:(n + 1) * D] = 1.0
        esl3[64 + n, n * D:(n + 1) * D] = 1.0
    w["esl2"] = bf(esl2)
    w["esl3"] = bf(esl3)
    w["ident"] = bf(np.eye(D))
    w["epsv"] = np.full((D, 1), EPS, dtype=np.float32)
    w["zerov"] = np.zeros((D, 1), dtype=np.float32)

    assert np.allclose(inp["proj_ln_g"], 1) and np.allclose(inp["proj_ln_b"], 0)
    assert np.allclose(inp["attn_ln_g"], 1) and np.allclose(inp["attn_ln_b"], 0)
    assert np.allclose(inp["ffn_ln_g"], 1) and np.allclose(inp["ffn_ln_b"], 0)
    assert np.allclose(inp["in_proj_b"], 0)
    return w


WEIGHT_SPECS = {
    "wspT": ((D, 10 * D), BF16), "wgfT": ((D, 2 * D), BF16),
    "bc": ((D, NB), F32), "emb": ((D, NB), F32),
    "wqT": ((D, NB * D), BF16), "wkT": ((D, NB * D), BF16),
    "wvT": ((D, NB * D), BF16), "owT": ((D, NB * D), BF16),
    "wovT": ((D, NB * D), BF16),
    "ob2": ((D, NB), F32),
    "w1T": ((D, NB * FFN), BF16), "b1": ((D, NB * 2), F32),
    "w2T": ((D, NB * 2 * D), BF16), "b2c": ((D, NB), F32),
    "gwT": ((D, NB * NB), BF16), "gateb": ((NB, 1), F32),
    "onesT": ((D, D), BF16), "hsel": ((D, D), BF16),
    "svsel": ((D, NB * NB), BF16),
    "ones3": ((NB, D), BF16), "esel": ((NB, NB * D), BF16),
    "esl2": ((32 + NB, NB * D), BF16), "esl3": ((64 + NB, NB * D), BF16),
    "ident": ((D, D), BF16),
    "epsv": ((D, 1), F32), "zerov": ((D, 1), F32),
}


def build_program(Bc, repeat=1):
    nc = bass.Bass()
    # pre-transposed feature-major inputs in HBM
    xsp = nc.dram_tensor("xspT", [D, 10, Bc], BF16, kind="ExternalInput")
    xg = nc.dram_tensor("xgT", [D, Bc], BF16, kind="ExternalInput")
    xf = nc.dram_tensor("xfT", [D, Bc], BF16, kind="ExternalInput")
    wd = {k: nc.dram_tensor(k, list(s[0]), s[1], kind="ExternalInput")
          for k, s in WEIGHT_SPECS.items()}
    out = nc.dram_tensor("outT", [D, Bc], BF16, kind="ExternalOutput")

    nblk = Bc // BLK
    assert Bc % BLK == 0

    with TileContext(nc) as tc, nc.allow_low_precision(reason="bf16 kernel"):
        with (
            tc.tile_pool(name="wp", bufs=1) as wp,
            tc.tile_pool(name="xin", bufs=2) as xin,
            tc.tile_pool(name="work", bufs=2) as wk_,
            tc.tile_pool(name="ps", bufs=8, space="PSUM") as psp,
        ):
            W = {}
            for k, s in WEIGHT_SPECS.items():
                W[k] = wp.tile(list(s[0]), s[1], tag=k, name=k)
                nc.gpsimd.dma_start(W[k][:], wd[k][:])
            ident = W["ident"]
            # shared LN-stats scratch: the 3 pipeline stages land their
            # Ln outputs at partition offsets 0/32/64 of one tile, so a
            # single Exp finishes all three rsqrt computations per tick
            lnvp = wp.tile([64 + NB, BLK], F32, tag="lnvp", name="lnvp")
            nc.vector.memset(lnvp[:], 0.0)

            def mm(out_ap, lhsT, rhs, start=True, stop=True):
                for h in range(BLK // MMN):
                    nc.tensor.matmul(out_ap[:, h * MMN:(h + 1) * MMN], lhsT,
                                     rhs[:, h * MMN:(h + 1) * MMN],
                                     start=start, stop=stop)

            def phase0(b):
                r0 = (b % nblk) * BLK
                st = {}
                for half, nmh in ((0, "xspA"), (1, "xspB")):
                    xt = xin.tile([D, 5 * BLK], BF16, tag="xspT", bufs=3,
                                  name=nmh)
                    nc.sync.dma_start(
                        xt[:].rearrange("p (c n) -> p c n", c=5),
                        xsp[:, 5 * half:5 * (half + 1), r0:r0 + BLK])
                    st[nmh] = xt
                st["xgT"] = xin.tile([D, BLK], BF16, tag="xgT", name="xgT")
                nc.sync.dma_start(st["xgT"][:], xg[:, r0:r0 + BLK])
                st["xfT"] = xin.tile([D, BLK], BF16, tag="xfT", name="xfT")
                nc.sync.dma_start(st["xfT"][:], xf[:, r0:r0 + BLK])
                return st

            NH = BLK // MMN   # psum halves per logical [D, BLK] tile

            def hmm(name, pairs, parts=D, poff=0):
                """Accumulating matmul into NH independent psum half tiles.
                pairs: [(lhsT_ap, full-width SBUF rhs AP)] accumulated.
                Emits half 0 fully, then half 1 (so half-0 consumers can
                start while half 1 accumulates).  Returns list of halves."""
                halves = []
                for h in range(NH):
                    pt = psp.tile([D, MMN], F32, tag="ps", name=f"{name}_{h}")
                    ap = pt[poff:poff + parts, :] if parts != D else pt[:]
                    for i, (lhsT, rhs) in enumerate(pairs):
                        nc.tensor.matmul(ap, lhsT,
                                         rhs[:, h * MMN:(h + 1) * MMN],
                                         start=(i == 0),
                                         stop=(i == len(pairs) - 1))
                    halves.append((pt, ap))
                return halves

            def for_halves(halves, sbuf_op):
                """sbuf_op(h, lo, hi, psum_ap) for each half."""
                for h, (pt, ap) in enumerate(halves):
                    sbuf_op(h, h * MMN, (h + 1) * MMN, ap)

            def stage_ln(sv_halves, off):
                """per-stage Ln of mean-squares into lnvp rows
                [off, off+NB); a single Exp per tick (in pS) finishes
                rsqrt for all three stages at once."""
                for_halves(sv_halves, lambda h, lo, hi, ap:
                           nc.scalar.activation(
                               lnvp[off:off + NB, lo:hi], ap, AF.Ln,
                               bias=W["epsv"][off:off + NB, 0:1]))

            def bcast_apply(rbca, stage, n, src_sb, out_sb, name):
                """out = src * broadcast(rbca row of stage/branch), per
                half.  Stage s reads rbca rows [0, 32*s+NB) against a
                zero-padded selector so only row 32*s+n contributes."""
                selw = (W["esel"], W["esl2"], W["esl3"])[stage]
                rows = 32 * stage + NB
                sel = selw[:rows, n * D:(n + 1) * D]
                for h in range(NH):
                    lo, hi = h * MMN, (h + 1) * MMN
                    rbb = psp.tile([D, MMN], F32, tag="ps",
                                   name=f"{name}_{h}")
                    nc.tensor.matmul(rbb[:], sel, rbca[:rows, lo:hi],
                                     start=True, stop=True)
                    nc.vector.tensor_tensor(out_sb[:, lo:hi], src_sb[:, lo:hi],
                                            rbb[:], AL.mult)

            def p1a(st):
                """projections + drains + squares + LN1 stats/rsqrt"""
                zh = []
                zh.append(hmm("zsp", [
                    (W["wspT"][:, c * D:(c + 1) * D],
                     st["xspA" if c < 5 else "xspB"][:, (c % 5) * BLK:
                                                     (c % 5 + 1) * BLK])
                    for c in range(10)]))
                zh.append(hmm("zg", [(W["wgfT"][:, 0:D], st["xgT"][:])]))
                zh.append(hmm("zf", [(W["wgfT"][:, D:2 * D], st["xfT"][:])]))
                zsb, sq = [], []
                for n in range(NB):
                    z_sb = wk_.tile([D, BLK], BF16, tag=f"zsb{n}", bufs=2)
                    for_halves(zh[n], lambda h, lo, hi, ap:
                               nc.scalar.activation(z_sb[:, lo:hi], ap,
                                                    AF.Identity,
                                                    bias=W["bc"][:, n:n + 1]))
                    zsb.append(z_sb)
                    s_ = wk_.tile([D, BLK], BF16, tag="sq1", bufs=1)
                    nc.vector.tensor_tensor(s_[:], z_sb[:], z_sb[:], AL.mult)
                    sq.append(s_)
                sv = hmm("sv1", [(W["svsel"][:, NB * n:NB * (n + 1)],
                                  sq[n][:]) for n in range(NB)], parts=NB)
                stage_ln(sv, 0)
                st["zsb"] = zsb

            def p1b(st, rbca):
                """LN1 broadcast + apply + emb + dP"""
                zsb = st.pop("zsb")
                P = []
                for n in range(NB):
                    p_ = wk_.tile([D, BLK], BF16, tag=f"P{n}")
                    bcast_apply(rbca, 0, n, zsb[n], p_, f"rbb1_{n}")
                    nc.vector.tensor_scalar_add(p_[:], p_[:],
                                                W["emb"][:, n:n + 1])
                    P.append(p_)
                st["P"] = P
                dP = []
                for n in range(NB):
                    s0, s1 = KV_IDX[n]
                    dp = wk_.tile([D, BLK], BF16, tag=f"dP{n}", bufs=2)
                    nc.vector.tensor_tensor(dp[:], P[s0][:], P[s1][:],
                                            AL.subtract)
                    dP.append(dp)
                st["dP"] = dP

            def p2a(st):
                """q/dk matmuls, q drain, score product"""
                P, dP = st["P"], st["dP"]
                t0 = {}
                for n in range(NB):
                    qh = hmm(f"q{n}", [(W["wqT"][:, n * D:(n + 1) * D],
                                        P[n][:])])
                    dkh = hmm(f"dk{n}", [(W["wkT"][:, n * D:(n + 1) * D],
                                          dP[n][:])])
                    q_sb = wk_.tile([D, BLK], BF16, tag="qsb", bufs=2,
                                    name=f"qsb{n}")
                    for_halves(qh, lambda h, lo, hi, ap:
                               nc.scalar.activation(q_sb[:, lo:hi], ap,
                                                    AF.Copy))
                    t0[n] = wk_.tile([D, BLK], BF16, tag="t0", bufs=2,
                                     name=f"t0{n}")
                    for_halves(dkh, lambda h, lo, hi, ap:
                               nc.vector.tensor_tensor(t0[n][:, lo:hi],
                                                       q_sb[:, lo:hi], ap,
                                                       AL.mult))
                st["t0"] = t0

            def p2b(st):
                """dv/score-bcast matmuls, tanh, fused (tanh+1)*dv"""
                dP = st["dP"]
                t0 = st.pop("t0")
                tp = {}
                for n in range(NB):
                    dvh = hmm(f"dv{n}", [(W["wvT"][:, n * D:(n + 1) * D],
                                          dP[n][:])])
                    dh = hmm(f"d{n}", [(W["hsel"][:], t0[n][:])])
                    th = wk_.tile([D, BLK], BF16, tag="th", bufs=2,
                                  name=f"th{n}")
                    for_halves(dh, lambda h, lo, hi, ap:
                               nc.scalar.activation(th[:, lo:hi], ap, AF.Tanh,
                                                    bias=W["zerov"][:, 0:1],
                                                    scale=0.5 * ISQ))
                    tp[n] = wk_.tile([D, BLK], BF16, tag="tp", bufs=2,
                                     name=f"tp{n}")
                    for_halves(dvh, lambda h, lo, hi, ap:
                               nc.vector.scalar_tensor_tensor(
                                   tp[n][:, lo:hi], th[:, lo:hi], 1.0, ap,
                                   AL.add, AL.mult))
                st["tp"] = tp

            def p2c(st):
                """attention out + residual (PE-folded) + LN2 stats/rsqrt"""
                P = st["P"]
                tp = st.pop("tp")
                us, sqs = [], []
                for n in range(NB):
                    s0, s1 = KV_IDX[n]
                    pairs = [(W["owT"][:, n * D:(n + 1) * D], tp[n][:]),
                             (W["wovT"][:, n * D:(n + 1) * D], P[s1][:])]
                    if n > 0:
                        pairs.append((ident[:], P[n][:]))
                    oh = hmm(f"o{n}", pairs)
                    u = wk_.tile([D, BLK], BF16, tag=f"u{n}", bufs=2)
                    if n > 0:
                        # ACT drain (+bias) with the +P residual PE-folded
                        for_halves(oh, lambda h, lo, hi, ap:
                                   nc.scalar.activation(
                                       u[:, lo:hi], ap, AF.Identity,
                                       bias=W["ob2"][:, n:n + 1]))
                    else:
                        # ACT is the HW-max engine (table-switch cost not in
                        # the sim); branch 0 takes the DVE STT path instead
                        for_halves(oh, lambda h, lo, hi, ap:
                                   nc.vector.scalar_tensor_tensor(
                                       u[:, lo:hi], ap,
                                       W["ob2"][:, n:n + 1],
                                       P[n][:, lo:hi], AL.add, AL.add))
                    us.append(u)
                    s_ = wk_.tile([D, BLK], BF16, tag="sq2", bufs=1)
                    nc.vector.tensor_tensor(s_[:], u[:], u[:], AL.mult)
                    sqs.append(s_)
                sv = hmm("sv2", [(W["svsel"][:, NB * n:NB * (n + 1)],
                                  sqs[n][:]) for n in range(NB)],
                         parts=NB, poff=32)
                stage_ln(sv, 32)
                st["us"] = us

            def p2d(st, rbca):
                """LN2 broadcast + apply"""
                us = st.pop("us")
                x1 = []
                for n in range(NB):
                    # x1 lives 3 ticks: made here, read by p3a and p3b1
                    x1n = wk_.tile([D, BLK], BF16, tag=f"x1{n}", bufs=3)
                    bcast_apply(rbca, 1, n, us[n], x1n, f"rbb2_{n}")
                    x1.append(x1n)
                st["x1"] = x1

            def phase3a(st):
                """FFN first half: W1 matmuls + gelu cluster."""
                x1 = st["x1"]
                hs_all = []
                for n in range(NB):
                    h_sb = []
                    for c in range(2):
                        hh = hmm(f"h{n}_{c}",
                                 [(W["w1T"][:, n * FFN + c * D:
                                            n * FFN + (c + 1) * D],
                                   x1[n][:])])
                        hs_ = wk_.tile([D, BLK], BF16, tag=f"hsb{n}_{c}",
                                       bufs=1)
                        for_halves(hh, lambda h, lo, hi, ap:
                                   nc.scalar.activation(
                                       hs_[:, lo:hi], ap, AF.Gelu,
                                       bias=W["b1"][:, 2 * n + c:
                                                    2 * n + c + 1]))
                        h_sb.append(hs_)
                    hs_all.append(h_sb)
                st["hs"] = hs_all

            def p3b1(st):
                """FFN second half + residual + LN3 stats/rsqrt"""
                x1 = st["x1"]
                x2ps, sqs = [], []
                for n in range(NB):
                    h_sb = st["hs"][n]
                    fh = hmm(f"f{n}",
                             [(W["w2T"][:, (2 * n) * D:(2 * n + 1) * D],
                               h_sb[0][:]),
                              (W["w2T"][:, (2 * n + 1) * D:(2 * n + 2) * D],
                               h_sb[1][:]),
                              (ident[:], x1[n][:])])
                    x2p = wk_.tile([D, BLK], BF16, tag=f"x2p{n}", bufs=2)
                    for_halves(fh, lambda h, lo, hi, ap:
                               nc.scalar.activation(x2p[:, lo:hi], ap,
                                                    AF.Identity,
                                                    bias=W["b2c"][:, n:n + 1]))
                    x2ps.append(x2p)
                    s_ = wk_.tile([D, BLK], BF16, tag="sq3", bufs=1)
                    nc.vector.tensor_tensor(s_[:], x2p[:], x2p[:], AL.mult)
                    sqs.append(s_)
                sv = hmm("sv3", [(W["svsel"][:, NB * n:NB * (n + 1)],
                                  sqs[n][:]) for n in range(NB)],
                         parts=NB, poff=64)
                stage_ln(sv, 64)
                st["x2ps"] = x2ps

            def p3b2(st, rbca):
                """LN3 broadcast + apply"""
                x2ps = st.pop("x2ps")
                st.pop("hs")
                x2 = []
                for n in range(NB):
                    x2n = wk_.tile([D, BLK], BF16, tag=f"x2{n}")
                    bcast_apply(rbca, 2, n, x2ps[n], x2n, f"rbb3_{n}")
                    x2.append(x2n)
                st["x2"] = x2

            def phase4(st, b):
                r0 = (b % nblk) * BLK
                x2 = st["x2"]
                gh = hmm("g", [(W["gwT"][:, n * NB:(n + 1) * NB], x2[n][:])
                               for n in range(NB)], parts=NB)
                e_sb = wk_.tile([NB, BLK], BF16, tag="esb", bufs=1)
                for_halves(gh, lambda h, lo, hi, ap:
                           nc.scalar.activation(e_sb[:, lo:hi], ap, AF.Exp,
                                                bias=W["gateb"][:NB, 0:1]))
                zbh = hmm("zb", [(W["ones3"][:NB, :], e_sb[:])])
                rz = wk_.tile([D, BLK], BF16, tag="rz", bufs=1)
                for_halves(zbh, lambda h, lo, hi, ap:
                           nc.vector.reciprocal(rz[:, lo:hi], ap))
                mns = []
                for n in range(NB):
                    ebh = hmm(f"eb{n}", [(W["esel"][:NB, n * D:(n + 1) * D],
                                          e_sb[:])])
                    mn = wk_.tile([D, BLK], BF16, tag=f"mn{n}", bufs=1)
                    for_halves(ebh, lambda h, lo, hi, ap:
                               nc.vector.tensor_tensor(mn[:, lo:hi],
                                                       x2[n][:, lo:hi], ap,
                                                       AL.mult))
                    mns.append(mn)
                acc = wk_.tile([D, BLK], BF16, tag="macc", bufs=1)
                nc.vector.tensor_tensor(acc[:], mns[0][:], mns[1][:], AL.add)
                acc2 = wk_.tile([D, BLK], BF16, tag="macc2", bufs=1)
                nc.vector.tensor_tensor(acc2[:], acc[:], mns[2][:], AL.add)
                fused = wk_.tile([D, BLK], BF16, tag="fused", bufs=1)
                nc.vector.tensor_tensor(fused[:], acc2[:], rz[:], AL.mult)
                nc.gpsimd.dma_start(out[:, r0:r0 + BLK], fused[:])

            # sub-phase interleaved emission, 6 blocks in flight; psum is
            # 8 independent [D, MMN] half-tile slots.  ACT table sets:
            # gelu+tanh (gelu_and_others) at the tick head, all Ln/Exp
            # users after -> 2 table switches per tick.
            total = nblk * repeat
            bstate = {}
            rbca_prev = rbca_cur = None
            marks = PHASE_MARKS
            marks.clear()

            def _run(label, fn, *a):
                i0 = len(nc.cur_bb.bb.instructions)
                fn(*a)
                i1 = len(nc.cur_bb.bb.instructions)
                marks.append((label, [x.name for x in
                                      nc.cur_bb.bb.instructions[i0:i1]]))

            for t in range(total + 8):
                if t < total:
                    bstate[t] = None
                    _run("p0", lambda tt=t: bstate.__setitem__(tt, phase0(tt)))
                if 0 <= t - 5 < total:
                    _run("p3a", phase3a, bstate[t - 5])
                if 0 <= t - 3 < total:
                    _run("p2a", p2a, bstate[t - 3])
                    _run("p2b", p2b, bstate[t - 3])
                if 0 <= t - 1 < total:
                    _run("p1a", p1a, bstate[t - 1])
                if 0 <= t - 3 < total:
                    _run("p2c", p2c, bstate[t - 3])
                if 0 <= t - 7 < total:
                    _run("p4", phase4, bstate[t - 7], t - 7)
                    bstate.pop(t - 7)
                if 0 <= t - 5 < total:
                    _run("p3b1", p3b1, bstate[t - 5])
                # apply sections ordered by next-tick consumer priority:
                # p2d feeds next tick's head (p3a), p1b the 2nd section
                # (p2a), p3b2 the tail (p4)
                if 0 <= t - 4 < total:
                    _run("p2d", p2d, bstate[t - 4], rbca_prev)
                if 0 <= t - 2 < total:
                    _run("p1b", p1b, bstate[t - 2], rbca_prev)
                if 0 <= t - 6 < total:
                    _run("p3b2", p3b2, bstate[t - 6], rbca_prev)
                if any(0 <= t - k < total for k in (1, 3, 5)):
                    # one Exp finishes rsqrt for all three LN stages
                    rbca_cur = wk_.tile([64 + NB, BLK], BF16, tag="rbca",
                                        bufs=2, name=f"rbca{t % 2}")
                    i0 = len(nc.cur_bb.bb.instructions)
                    nc.scalar.activation(rbca_cur[:], lnvp[:], AF.Exp,
                                         scale=-0.5,
                                         bias=W["zerov"][0:64 + NB, 0:1])
                    marks.append(("pS", [x.name for x in
                                         nc.cur_bb.bb.instructions[i0:]]))
                rbca_prev = rbca_cur
    _fix_wait_overflow(nc)
    return nc


def prep_x(inputs, Bc=None):
    """Host-side: cast to bf16 and pre-transpose into feature-major HBM
    layouts."""
    xsp = np.ascontiguousarray(inputs["x_spatial"]).astype(NPBF)
    B = xsp.shape[0]
    xspT = np.ascontiguousarray(xsp.reshape(B, 10, D).transpose(2, 1, 0))
    xgT = np.ascontiguousarray(inputs["x_gradient"].T.astype(NPBF))
    xfT = np.ascontiguousarray(inputs["x_frequency"].T.astype(NPBF))
    return {"xspT": xspT, "xgT": xgT, "xfT": xfT}


def kernel(**inputs):
    _patch_tile_drain()
    B = inputs["x_spatial"].shape[0]
    Bc = B // NCORES
    w = prep_weights(inputs)
    xb = prep_x(inputs)
    nc = build_program(Bc)
    in_maps = []
    for c in range(NCORES):
        m = dict(w)
        m["xspT"] = np.ascontiguousarray(xb["xspT"][:, :, c * Bc:(c + 1) * Bc])
        m["xgT"] = np.ascontiguousarray(xb["xgT"][:, c * Bc:(c + 1) * Bc])
        m["xfT"] = np.ascontiguousarray(xb["xfT"][:, c * Bc:(c + 1) * Bc])
        in_maps.append(m)
    res = run_bass_kernel_spmd(nc, in_maps, list(range(NCORES)))
    outs = [res.results[c]["outT"] for c in range(NCORES)]
    full = np.concatenate([o.T for o in outs], axis=0)
    return np.ascontiguousarray(full.astype(np.float32))


# revision 39
# speedup vs baseline: 1.8007x; 1.3869x over previous
"""Trainium2 Bass kernel for nn_CMAF (cross-modal attention fusion block).

Layout: feature-major activations on-chip — every tile is
[128 features (partitions) x 1024 samples (free)], so all matmuls are
weight-stationary bf16 with the batch as the moving free dimension.
Inputs are pre-transposed host-side into feature-major HBM layouts, so
device DMA is fully contiguous (no DMA-transpose).

Engine-balance design (ACT/DVE were the baseline bottleneck):
 - LN stats (sum of squares) for all 3 branches land in ONE [3,1024]
   PSUM tile; Ln+Exp (rsqrt) run once per LN stage on that compact tile
   instead of per-branch full tiles; per-branch ones-matmuls broadcast
   the result back to 128 partitions (PE pump is cheap).
 - Residual adds (u = o + P, x2p = f + x1) are folded into the PE as
   identity-matrix accumulation matmuls, killing 1x-rate STT DVE ops.
 - Wo@v1 is folded host-side into Wov = (C Wo) Wv and accumulated into
   the same PSUM as Wo@tp, killing the tpv add.
 - The 2-way attention softmax collapses to division by (1+exp(-d/sqrt(dh)))
   done as a single DVE tensor_tensor divide straight from PSUM.
 - Gelu ACT ops are clustered at alternating head/tail of the pipeline
   tick so the ACT table set (gelu vs natural_log_exp) switches once per
   block on average instead of twice.

Data parallel over 8 NeuronCores: 8192 samples each.
"""

import numpy as np
import ml_dtypes

import concourse.bass as bass
import concourse.mybir as mybir
from concourse.tile import TileContext
from concourse.vector_clock import ScopedClock
from concourse.bass_utils import run_bass_kernel_spmd

F32 = mybir.dt.float32
BF16 = mybir.dt.bfloat16
AL = mybir.AluOpType
AF = mybir.ActivationFunctionType
NPBF = ml_dtypes.bfloat16

D = 128
SP = 1280
FFN = 256
NB = 3
DH = 32
KV_IDX = ((1, 2), (0, 2), (0, 1))
NCORES = 8
BLK = 1024
MMN = 512
EPS = 1e-5
ISQ = float(1.0 / np.sqrt(DH))

# tuning flags
IDENT_FOLD = True      # residual adds via identity matmuls on PE

# filled by build_program: [(phase_label, [instruction names]), ...]
PHASE_MARKS = []


def _patch_tile_drain():
    """walrus here rejects >4 sem waits on one instruction; Tile's tail
    drain carries one wait per logical proc.  Re-emit them as standalone
    wait_ge instructions ahead of the drain."""
    TC = TileContext
    if getattr(TC, "_drain_patched", False):
        return

    def patched(self, tick_clock, wait_clock):
        nop_inst = self.nc.sync.nop()
        wait_clock.add_sem_waits(
            nop_inst.ins, ScopedClock({None: tick_clock.global_clock})
        )
        d = nop_inst.ins
        si = d.sync_info
        waits = list(si.on_wait) if si is not None else []
        if len(waits) > 4:
            si.on_wait = []
            d.sync_info = si
            name2sem = {s.name: s for s in self.sems.allocated().values()}
            for w in waits:
                sem = name2sem.get(w.ant_name)
                if sem is None:
                    raise RuntimeError(f"drain patch: unknown sem {w.ant_name}")
                self.nc.sync.wait_ge(sem, w.wait_value)
        self.nc.sync.drain()
        self.nc.all_engine_barrier()
        popped = self.nc._tile_sem_poison_stack.pop()
        assert popped is self._sem_poison
        self.nc.clear_and_free_semaphores(list(self.sems.allocated().values()))
        self.nc.all_engine_barrier()

    TC._drain_and_barrier = patched
    TC._drain_patched = True


def _fix_wait_overflow(nc):
    """walrus enforces per-opcode caps on sync-wait commands attached to
    one instruction (DmaTransposeAnt: 1, others: ~4).  Move the excess
    onto same-engine NOPs inserted immediately before the instruction."""
    LIMITS = {}
    DEFAULT_LIM = 1
    for fn in nc.m.functions:
        for bb in fn.blocks:
            insts = list(bb.instructions)
            out = []
            changed = False
            for inst in insts:
                si = getattr(inst, "sync_info", None)
                w = list(si.on_wait) if si is not None and si.on_wait else []
                lim = LIMITS.get(type(inst).__name__, DEFAULT_LIM)
                if len(w) > lim:
                    excess = w[lim:]
                    keep = w[:lim]
                    eng = nc.engines[inst.engine]
                    nops = []
                    for i in range(0, len(excess), 1):
                        chunk = excess[i:i + 1]
                        nop_bi = eng.nop()
                        nop_inst = nop_bi.ins
                        cb = nc.cur_bb.bb
                        cb.instructions = [x for x in cb.instructions
                                           if x.name != nop_inst.name]
                        import bass_rust
                        nop_inst.sync_info = bass_rust.SyncInfo(
                            on_wait=chunk, on_update=[])
                        nops.append(nop_inst)
                    si.on_wait = keep
                    inst.sync_info = si
                    out.extend(nops)
                    changed = True
                out.append(inst)
            if changed:
                bb.instructions = out


def prep_weights(inp):
    """Host-side prep of all weights into SBUF layouts. bf16 for matmul
    operands, fp32 for per-partition bias vectors."""
    f64 = np.float64
    C = np.eye(D, dtype=f64) - 1.0 / D

    def bf(a):
        return np.ascontiguousarray(a.astype(np.float32)).astype(NPBF)

    def f32(a):
        return np.ascontiguousarray(a, dtype=np.float32)

    w = {}
    wsp = C @ inp["proj_w_spatial"].astype(f64)            # [128,1280]
    w["wspT"] = bf(np.transpose(wsp.reshape(D, 10, D), (2, 1, 0)).reshape(D, 10 * D))
    wgf = np.stack([C @ inp["proj_w_gf"][i].astype(f64) for i in range(2)])
    w["wgfT"] = bf(np.transpose(wgf, (2, 0, 1)).reshape(D, 2 * D))
    w["bc"] = f32(C @ inp["proj_b"].astype(f64).T)         # [128,3]
    w["emb"] = f32(inp["mod_emb"].T)

    ipw = inp["in_proj_w"].astype(f64)                     # [3, 384, 128]
    wq, wk, wv = ipw[:, :D], ipw[:, D:2 * D], ipw[:, 2 * D:]
    w["wqT"] = bf(np.transpose(wq, (2, 0, 1)).reshape(D, NB * D))
    w["wkT"] = bf(np.transpose(wk, (2, 0, 1)).reshape(D, NB * D))
    w["wvT"] = bf(np.transpose(wv, (2, 0, 1)).reshape(D, NB * D))
    # negated copies: dk/dv are computed as W@P_s0 + (-W)@P_s1 on the PE,
    # eliminating the dP tiles (and their DVE subtracts) entirely
    w["wknT"] = bf(-np.transpose(wk, (2, 0, 1)).reshape(D, NB * D))
    w["wvnT"] = bf(-np.transpose(wv, (2, 0, 1)).reshape(D, NB * D))
    ow = np.stack([C @ inp["out_proj_w"][n].astype(f64) for n in range(NB)])
    # 0.5x fold: attention prob a = (1+tanh(d/(2 sqrt(dh))))/2, the 1/2 is
    # folded here so tp = (tanh+1)*dv feeds Wo directly
    w["owT"] = bf(0.5 * np.transpose(ow, (2, 0, 1)).reshape(D, NB * D))
    # Wov[n] = (C @ Wo[n]) @ Wv[n] : folds the v1 path into one matmul
    wov = np.stack([ow[n] @ wv[n] for n in range(NB)])
    w["wovT"] = bf(np.transpose(wov, (2, 0, 1)).reshape(D, NB * D))
    ob2 = np.stack([
        C @ inp["out_proj_b"][n].astype(f64)
        - inp["mod_emb"][n].astype(f64).mean()
        for n in range(NB)])
    w["ob2"] = f32(ob2.T)

    w1 = inp["ffn_w1"].astype(f64)                         # [3, 256, 128]
    w["w1T"] = bf(np.transpose(w1, (2, 0, 1)).reshape(D, NB * FFN))
    w["b1"] = f32(inp["ffn_b1"].reshape(NB * 2, D).T)      # [128, 6]
    w2 = np.stack([C @ inp["ffn_w2"][n].astype(f64) for n in range(NB)])
    w2c = w2.reshape(NB, D, 2, D)                          # [n, j, c, p]
    w["w2T"] = bf(np.transpose(w2c, (3, 0, 2, 1)).reshape(D, NB * 2 * D))
    b2c = np.stack([C @ inp["ffn_b2"][n].astype(f64) for n in range(NB)])
    w["b2c"] = f32(b2c.T)

    gw = inp["gate_w"].astype(f64).reshape(NB, NB, D)      # [j, n, p]
    w["gwT"] = bf(np.transpose(gw, (2, 1, 0)).reshape(D, NB * NB))
    w["gateb"] = f32(inp["gate_b"].reshape(NB, 1))

    w["onesT"] = bf(np.full((D, D), 1.0 / D))
    svsel = np.zeros((D, NB * NB), dtype=np.float32)
    for n in range(NB):
        svsel[:, NB * n + n] = 1.0 / D
    w["svsel"] = bf(svsel)
    hs = np.zeros((D, D), dtype=np.float32)
    for h in range(4):
        hs[h * DH:(h + 1) * DH, h * DH:(h + 1) * DH] = 1.0
    w["hsel"] = bf(hs)
    w["ones3"] = bf(np.ones((NB, D)))
    esel = np.zeros((NB, NB * D), dtype=np.float32)
    for n in range(NB):
        esel[n, n * D:(n + 1) * D] = 1.0
    w["esel"] = bf(esel)
    esl2 = np.zeros((32 + NB, NB * D), dtype=np.float32)
    esl3 = np.zeros((64 + NB, NB * D), dtype=np.float32)
    for n in range(NB):
        esl2[32 + n, n * D:(n + 1) * D] = 1.0
        esl3[64 + n, n * D:(n + 1) * D] = 1.0
    w["esl2"] = bf(esl2)
    w["esl3"] = bf(esl3)
    w["ident"] = bf(np.eye(D))
    w["epsv"] = np.full((D, 1), EPS, dtype=np.float32)
    w["zerov"] = np.zeros((D, 1), dtype=np.float32)

    assert np.allclose(inp["proj_ln_g"], 1) and np.allclose(inp["proj_ln_b"], 0)
    assert np.allclose(inp["attn_ln_g"], 1) and np.allclose(inp["attn_ln_b"], 0)
    assert np.allclose(inp["ffn_ln_g"], 1) and np.allclose(inp["ffn_ln_b"], 0)
    assert np.allclose(inp["in_proj_b"], 0)
    return w


WEIGHT_SPECS = {
    "wspT": ((D, 10 * D), BF16), "wgfT": ((D, 2 * D), BF16),
    "bc": ((D, NB), F32), "emb": ((D, NB), F32),
    "wqT": ((D, NB * D), BF16), "wkT": ((D, NB * D), BF16),
    "wvT": ((D, NB * D), BF16), "owT": ((D, NB * D), BF16),
    "wknT": ((D, NB * D), BF16), "wvnT": ((D, NB * D), BF16),
    "wovT": ((D, NB * D), BF16),
    "ob2": ((D, NB), F32),
    "w1T": ((D, NB * FFN), BF16), "b1": ((D, NB * 2), F32),
    "w2T": ((D, NB * 2 * D), BF16), "b2c": ((D, NB), F32),
    "gwT": ((D, NB * NB), BF16), "gateb": ((NB, 1), F32),
    "onesT": ((D, D), BF16), "hsel": ((D, D), BF16),
    "svsel": ((D, NB * NB), BF16),
    "ones3": ((NB, D), BF16), "esel": ((NB, NB * D), BF16),
    "esl2": ((32 + NB, NB * D), BF16), "esl3": ((64 + NB, NB * D), BF16),
    "ident": ((D, D), BF16),
    "epsv": ((D, 1), F32), "zerov": ((D, 1), F32),
}


def build_program(Bc, repeat=1):
    nc = bass.Bass()
    # pre-transposed feature-major inputs in HBM
    xsp = nc.dram_tensor("xspT", [D, 10, Bc], BF16, kind="ExternalInput")
    xg = nc.dram_tensor("xgT", [D, Bc], BF16, kind="ExternalInput")
    xf = nc.dram_tensor("xfT", [D, Bc], BF16, kind="ExternalInput")
    wd = {k: nc.dram_tensor(k, list(s[0]), s[1], kind="ExternalInput")
          for k, s in WEIGHT_SPECS.items()}
    out = nc.dram_tensor("outT", [D, Bc], BF16, kind="ExternalOutput")

    nblk = Bc // BLK
    assert Bc % BLK == 0

    with TileContext(nc) as tc, nc.allow_low_precision(reason="bf16 kernel"):
        with (
            tc.tile_pool(name="wp", bufs=1) as wp,
            tc.tile_pool(name="xin", bufs=2) as xin,
            tc.tile_pool(name="work", bufs=2) as wk_,
            tc.tile_pool(name="ps", bufs=8, space="PSUM") as psp,
        ):
            W = {}
            for k, s in WEIGHT_SPECS.items():
                W[k] = wp.tile(list(s[0]), s[1], tag=k, name=k)
                nc.gpsimd.dma_start(W[k][:], wd[k][:])
            ident = W["ident"]
            # shared LN-stats scratch: the 3 pipeline stages land their
            # Ln outputs at partition offsets 0/32/64 of one tile, so a
            # single Exp finishes all three rsqrt computations per tick
            lnvp = wp.tile([64 + NB, BLK], F32, tag="lnvp", name="lnvp")
            nc.vector.memset(lnvp[:], 0.0)

            def mm(out_ap, lhsT, rhs, start=True, stop=True):
                for h in range(BLK // MMN):
                    nc.tensor.matmul(out_ap[:, h * MMN:(h + 1) * MMN], lhsT,
                                     rhs[:, h * MMN:(h + 1) * MMN],
                                     start=start, stop=stop)

            def phase0(b):
                r0 = (b % nblk) * BLK
                st = {}
                for half, nmh in ((0, "xspA"), (1, "xspB")):
                    xt = xin.tile([D, 5 * BLK], BF16, tag="xspT", bufs=3,
                                  name=nmh)
                    nc.sync.dma_start(
                        xt[:].rearrange("p (c n) -> p c n", c=5),
                        xsp[:, 5 * half:5 * (half + 1), r0:r0 + BLK])
                    st[nmh] = xt
                st["xgT"] = xin.tile([D, BLK], BF16, tag="xgT", name="xgT")
                nc.sync.dma_start(st["xgT"][:], xg[:, r0:r0 + BLK])
                st["xfT"] = xin.tile([D, BLK], BF16, tag="xfT", name="xfT")
                nc.sync.dma_start(st["xfT"][:], xf[:, r0:r0 + BLK])
                return st

            NH = BLK // MMN   # psum halves per logical [D, BLK] tile

            def hmm(name, pairs, parts=D, poff=0):
                """Accumulating matmul into NH independent psum half tiles.
                pairs: [(lhsT_ap, full-width SBUF rhs AP)] accumulated.
                Emits half 0 fully, then half 1 (so half-0 consumers can
                start while half 1 accumulates).  Returns list of halves."""
                halves = []
                for h in range(NH):
                    pt = psp.tile([D, MMN], F32, tag="ps", name=f"{name}_{h}")
                    ap = pt[poff:poff + parts, :] if parts != D else pt[:]
                    for i, (lhsT, rhs) in enumerate(pairs):
                        nc.tensor.matmul(ap, lhsT,
                                         rhs[:, h * MMN:(h + 1) * MMN],
                                         start=(i == 0),
                                         stop=(i == len(pairs) - 1))
                    halves.append((pt, ap))
                return halves

            def for_halves(halves, sbuf_op):
                """sbuf_op(h, lo, hi, psum_ap) for each half."""
                for h, (pt, ap) in enumerate(halves):
                    sbuf_op(h, h * MMN, (h + 1) * MMN, ap)

            def stage_ln(sv_halves, off):
                """per-stage Ln of mean-squares into lnvp rows
                [off, off+NB); a single Exp per tick (in pS) finishes
                rsqrt for all three stages at once."""
                for_halves(sv_halves, lambda h, lo, hi, ap:
                           nc.scalar.activation(
                               lnvp[off:off + NB, lo:hi], ap, AF.Ln,
                               bias=W["epsv"][off:off + NB, 0:1]))

            def bcast_apply(rbca, stage, n, src_sb, out_sb, name):
                """out = src * broadcast(rbca row of stage/branch), per
                half.  Stage s reads rbca rows [0, 32*s+NB) against a
                zero-padded selector so only row 32*s+n contributes."""
                selw = (W["esel"], W["esl2"], W["esl3"])[stage]
                rows = 32 * stage + NB
                sel = selw[:rows, n * D:(n + 1) * D]
                for h in range(NH):
                    lo, hi = h * MMN, (h + 1) * MMN
                    rbb = psp.tile([D, MMN], F32, tag="ps",
                                   name=f"{name}_{h}")
                    nc.tensor.matmul(rbb[:], sel, rbca[:rows, lo:hi],
                                     start=True, stop=True)
                    nc.vector.tensor_tensor(out_sb[:, lo:hi], src_sb[:, lo:hi],
                                            rbb[:], AL.mult)

            def p1a(st):
                """projections + drains + squares + LN1 stats/rsqrt"""
                zh = []
                zh.append(hmm("zsp", [
                    (W["wspT"][:, c * D:(c + 1) * D],
                     st["xspA" if c < 5 else "xspB"][:, (c % 5) * BLK:
                                                     (c % 5 + 1) * BLK])
                    for c in range(10)]))
                zh.append(hmm("zg", [(W["wgfT"][:, 0:D], st["xgT"][:])]))
                zh.append(hmm("zf", [(W["wgfT"][:, D:2 * D], st["xfT"][:])]))
                zsb, sq = [], []
                for n in range(NB):
                    z_sb = wk_.tile([D, BLK], BF16, tag=f"zsb{n}", bufs=2)
                    for_halves(zh[n], lambda h, lo, hi, ap:
                               nc.scalar.activation(z_sb[:, lo:hi], ap,
                                                    AF.Identity,
                                                    bias=W["bc"][:, n:n + 1]))
                    zsb.append(z_sb)
                    s_ = wk_.tile([D, BLK], BF16, tag="sq1", bufs=1)
                    nc.vector.tensor_tensor(s_[:], z_sb[:], z_sb[:], AL.mult)
                    sq.append(s_)
                sv = hmm("sv1", [(W["svsel"][:, NB * n:NB * (n + 1)],
                                  sq[n][:]) for n in range(NB)], parts=NB)
                stage_ln(sv, 0)
                st["zsb"] = zsb

            def p1b(st, rbca):
                """LN1 broadcast + apply + emb + dP"""
                zsb = st.pop("zsb")
                P = []
                for n in range(NB):
                    p_ = wk_.tile([D, BLK], BF16, tag=f"P{n}")
                    bcast_apply(rbca, 0, n, zsb[n], p_, f"rbb1_{n}")
                    nc.vector.tensor_scalar_add(p_[:], p_[:],
                                                W["emb"][:, n:n + 1])
                    P.append(p_)
                st["P"] = P

            def p2a(st):
                """q/dk matmuls, q drain, score product"""
                P = st["P"]
                t0 = {}
                for n in range(NB):
                    s0, s1 = KV_IDX[n]
                    qh = hmm(f"q{n}", [(W["wqT"][:, n * D:(n + 1) * D],
                                        P[n][:])])
                    dkh = hmm(f"dk{n}",
                              [(W["wkT"][:, n * D:(n + 1) * D], P[s0][:]),
                               (W["wknT"][:, n * D:(n + 1) * D], P[s1][:])])
                    q_sb = wk_.tile([D, BLK], BF16, tag="qsb", bufs=2,
                                    name=f"qsb{n}")
                    for_halves(qh, lambda h, lo, hi, ap:
                               nc.scalar.activation(q_sb[:, lo:hi], ap,
                                                    AF.Copy))
                    t0[n] = wk_.tile([D, BLK], BF16, tag="t0", bufs=2,
                                     name=f"t0{n}")
                    for_halves(dkh, lambda h, lo, hi, ap:
                               nc.vector.tensor_tensor(t0[n][:, lo:hi],
                                                       q_sb[:, lo:hi], ap,
                                                       AL.mult))
                st["t0"] = t0

            def p2b(st):
                """dv/score-bcast matmuls, tanh, fused (tanh+1)*dv"""
                P = st["P"]
                t0 = st.pop("t0")
                tp = {}
                for n in range(NB):
                    s0, s1 = KV_IDX[n]
                    dvh = hmm(f"dv{n}",
                              [(W["wvT"][:, n * D:(n + 1) * D], P[s0][:]),
                               (W["wvnT"][:, n * D:(n + 1) * D], P[s1][:])])
                    dh = hmm(f"d{n}", [(W["hsel"][:], t0[n][:])])
                    th = wk_.tile([D, BLK], BF16, tag="th", bufs=2,
                                  name=f"th{n}")
                    for_halves(dh, lambda h, lo, hi, ap:
                               nc.scalar.activation(th[:, lo:hi], ap, AF.Tanh,
                                                    bias=W["zerov"][:, 0:1],
                                                    scale=0.5 * ISQ))
                    tp[n] = wk_.tile([D, BLK], BF16, tag="tp", bufs=2,
                                     name=f"tp{n}")
                    for_halves(dvh, lambda h, lo, hi, ap:
                               nc.vector.scalar_tensor_tensor(
                                   tp[n][:, lo:hi], th[:, lo:hi], 1.0, ap,
                                   AL.add, AL.mult))
                st["tp"] = tp

            def p2c(st):
                """attention out + residual (PE-folded) + LN2 stats/rsqrt"""
                P = st["P"]
                tp = st.pop("tp")
                us, sqs = [], []
                for n in range(NB):
                    s0, s1 = KV_IDX[n]
                    pairs = [(W["owT"][:, n * D:(n + 1) * D], tp[n][:]),
                             (W["wovT"][:, n * D:(n + 1) * D], P[s1][:])]
                    if n > 1:
                        pairs.append((ident[:], P[n][:]))
                    oh = hmm(f"o{n}", pairs)
                    u = wk_.tile([D, BLK], BF16, tag=f"u{n}", bufs=2)
                    if n > 1:
                        # ACT drain (+bias) with the +P residual PE-folded
                        for_halves(oh, lambda h, lo, hi, ap:
                                   nc.scalar.activation(
                                       u[:, lo:hi], ap, AF.Identity,
                                       bias=W["ob2"][:, n:n + 1]))
                    else:
                        # ACT is the HW-max engine (table-switch cost not in
                        # the sim); branch 0 takes the DVE STT path instead
                        for_halves(oh, lambda h, lo, hi, ap:
                                   nc.vector.scalar_tensor_tensor(
                                       u[:, lo:hi], ap,
                                       W["ob2"][:, n:n + 1],
                                       P[n][:, lo:hi], AL.add, AL.add))
                    us.append(u)
                    s_ = wk_.tile([D, BLK], BF16, tag="sq2", bufs=1)
                    nc.vector.tensor_tensor(s_[:], u[:], u[:], AL.mult)
                    sqs.append(s_)
                sv = hmm("sv2", [(W["svsel"][:, NB * n:NB * (n + 1)],
                                  sqs[n][:]) for n in range(NB)],
                         parts=NB, poff=32)
                stage_ln(sv, 32)
                st["us"] = us

            def p2d(st, rbca):
                """LN2 broadcast + apply"""
                us = st.pop("us")
                x1 = []
                for n in range(NB):
                    # x1 lives 3 ticks: made here, read by p3a and p3b1
                    x1n = wk_.tile([D, BLK], BF16, tag=f"x1{n}", bufs=3)
                    bcast_apply(rbca, 1, n, us[n], x1n, f"rbb2_{n}")
                    x1.append(x1n)
                st["x1"] = x1

            def phase3a(st):
                """FFN first half: W1 matmuls + gelu cluster."""
                x1 = st["x1"]
                hs_all = []
                for n in range(NB):
                    h_sb = []
                    for c in range(2):
                        hh = hmm(f"h{n}_{c}",
                                 [(W["w1T"][:, n * FFN + c * D:
                                            n * FFN + (c + 1) * D],
                                   x1[n][:])])
                        hs_ = wk_.tile([D, BLK], BF16, tag=f"hsb{n}_{c}",
                                       bufs=1)
                        for_halves(hh, lambda h, lo, hi, ap:
                                   nc.scalar.activation(
                                       hs_[:, lo:hi], ap, AF.Gelu,
                                       bias=W["b1"][:, 2 * n + c:
                                                    2 * n + c + 1]))
                        h_sb.append(hs_)
                    hs_all.append(h_sb)
                st["hs"] = hs_all

            def p3b1(st):
                """FFN second half + residual + LN3 stats/rsqrt"""
                x1 = st["x1"]
                x2ps, sqs = [], []
                for n in range(NB):
                    h_sb = st["hs"][n]
                    fh = hmm(f"f{n}",
                             [(W["w2T"][:, (2 * n) * D:(2 * n + 1) * D],
                               h_sb[0][:]),
                              (W["w2T"][:, (2 * n + 1) * D:(2 * n + 2) * D],
                               h_sb[1][:]),
                              (ident[:], x1[n][:])])
                    x2p = wk_.tile([D, BLK], BF16, tag=f"x2p{n}", bufs=2)
                    for_halves(fh, lambda h, lo, hi, ap:
                               nc.scalar.activation(x2p[:, lo:hi], ap,
                                                    AF.Identity,
                                                    bias=W["b2c"][:, n:n + 1]))
                    x2ps.append(x2p)
                    s_ = wk_.tile([D, BLK], BF16, tag="sq3", bufs=1)
                    nc.vector.tensor_tensor(s_[:], x2p[:], x2p[:], AL.mult)
                    sqs.append(s_)
                sv = hmm("sv3", [(W["svsel"][:, NB * n:NB * (n + 1)],
                                  sqs[n][:]) for n in range(NB)],
                         parts=NB, poff=64)
                stage_ln(sv, 64)
                st["x2ps"] = x2ps

            def p3b2(st, rbca):
                """LN3 broadcast + apply"""
                x2ps = st.pop("x2ps")
                st.pop("hs")
                x2 = []
                for n in range(NB):
                    x2n = wk_.tile([D, BLK], BF16, tag=f"x2{n}")
                    bcast_apply(rbca, 2, n, x2ps[n], x2n, f"rbb3_{n}")
                    x2.append(x2n)
                st["x2"] = x2

            def phase4(st, b):
                r0 = (b % nblk) * BLK
                x2 = st["x2"]
                gh = hmm("g", [(W["gwT"][:, n * NB:(n + 1) * NB], x2[n][:])
                               for n in range(NB)], parts=NB)
                e_sb = wk_.tile([NB, BLK], BF16, tag="esb", bufs=1)
                for_halves(gh, lambda h, lo, hi, ap:
                           nc.scalar.activation(e_sb[:, lo:hi], ap, AF.Exp,
                                                bias=W["gateb"][:NB, 0:1]))
                zbh = hmm("zb", [(W["ones3"][:NB, :], e_sb[:])])
                rz = wk_.tile([D, BLK], BF16, tag="rz", bufs=1)
                for_halves(zbh, lambda h, lo, hi, ap:
                           nc.vector.reciprocal(rz[:, lo:hi], ap))
                mns = []
                for n in range(NB):
                    ebh = hmm(f"eb{n}", [(W["esel"][:NB, n * D:(n + 1) * D],
                                          e_sb[:])])
                    mn = wk_.tile([D, BLK], BF16, tag=f"mn{n}", bufs=1)
                    for_halves(ebh, lambda h, lo, hi, ap:
                               nc.vector.tensor_tensor(mn[:, lo:hi],
                                                       x2[n][:, lo:hi], ap,
                                                       AL.mult))
                    mns.append(mn)
                acc = wk_.tile([D, BLK], BF16, tag="macc", bufs=1)
                nc.vector.tensor_tensor(acc[:], mns[0][:], mns[1][:], AL.add)
                acc2 = wk_.tile([D, BLK], BF16, tag="macc2", bufs=1)
                nc.vector.tensor_tensor(acc2[:], acc[:], mns[2][:], AL.add)
                fused = wk_.tile([D, BLK], BF16, tag="fused", bufs=1)
                nc.vector.tensor_tensor(fused[:], acc2[:], rz[:], AL.mult)
                nc.gpsimd.dma_start(out[:, r0:r0 + BLK], fused[:])

            # sub-phase interleaved emission, 6 blocks in flight; psum is
            # 8 independent [D, MMN] half-tile slots.  ACT table sets:
            # gelu+tanh (gelu_and_others) at the tick head, all Ln/Exp
            # users after -> 2 table switches per tick.
            total = nblk * repeat
            bstate = {}
            rbca_prev = rbca_cur = None
            marks = PHASE_MARKS
            marks.clear()

            def _run(label, fn, *a):
                i0 = len(nc.cur_bb.bb.instructions)
                fn(*a)
                i1 = len(nc.cur_bb.bb.instructions)
                marks.append((label, [x.name for x in
                                      nc.cur_bb.bb.instructions[i0:i1]]))

            for t in range(total + 8):
                if t < total:
                    bstate[t] = None
                    _run("p0", lambda tt=t: bstate.__setitem__(tt, phase0(tt)))
                if 0 <= t - 5 < total:
                    _run("p3a", phase3a, bstate[t - 5])
                if 0 <= t - 3 < total:
                    _run("p2a", p2a, bstate[t - 3])
                    _run("p2b", p2b, bstate[t - 3])
                if 0 <= t - 1 < total:
                    _run("p1a", p1a, bstate[t - 1])
                if 0 <= t - 3 < total:
                    _run("p2c", p2c, bstate[t - 3])
                if 0 <= t - 7 < total:
                    _run("p4", phase4, bstate[t - 7], t - 7)
                    bstate.pop(t - 7)
                if 0 <= t - 5 < total:
                    _run("p3b1", p3b1, bstate[t - 5])
                # apply sections ordered by next-tick consumer priority:
                # p2d feeds next tick's head (p3a), p1b the 2nd section
                # (p2a), p3b2 the tail (p4)
                if 0 <= t - 4 < total:
                    _run("p2d", p2d, bstate[t - 4], rbca_prev)
                if 0 <= t - 2 < total:
                    _run("p1b", p1b, bstate[t - 2], rbca_prev)
                if 0 <= t - 6 < total:
                    _run("p3b2", p3b2, bstate[t - 6], rbca_prev)
                if any(0 <= t - k < total for k in (1, 3, 5)):
                    # one Exp finishes rsqrt for all three LN stages
                    rbca_cur = wk_.tile([64 + NB, BLK], BF16, tag="rbca",
                                        bufs=2, name=f"rbca{t % 2}")
                    i0 = len(nc.cur_bb.bb.instructions)
                    nc.scalar.activation(rbca_cur[:], lnvp[:], AF.Exp,
                                         scale=-0.5,
                                         bias=W["zerov"][0:64 + NB, 0:1])
                    marks.append(("pS", [x.name for x in
                                         nc.cur_bb.bb.instructions[i0:]]))
                rbca_prev = rbca_cur
    _fix_wait_overflow(nc)
    return nc


def prep_x(inputs, Bc=None):
    """Host-side: cast to bf16 and pre-transpose into feature-major HBM
    layouts."""
    xsp = np.ascontiguousarray(inputs["x_spatial"]).astype(NPBF)
    B = xsp.shape[0]
    xspT = np.ascontiguousarray(xsp.reshape(B, 10, D).transpose(2, 1, 0))
    xgT = np.ascontiguousarray(inputs["x_gradient"].T.astype(NPBF))
    xfT = np.ascontiguousarray(inputs["x_frequency"].T.astype(NPBF))
    return {"xspT": xspT, "xgT": xgT, "xfT": xfT}


def kernel(**inputs):
    _patch_tile_drain()
    B = inputs["x_spatial"].shape[0]
    Bc = B // NCORES
    w = prep_weights(inputs)
    xb = prep_x(inputs)
    nc = build_program(Bc)
    in_maps = []
    for c in range(NCORES):
        m = dict(w)
        m["xspT"] = np.ascontiguousarray(xb["xspT"][:, :, c * Bc:(c + 1) * Bc])
        m["xgT"] = np.ascontiguousarray(xb["xgT"][:, c * Bc:(c + 1) * Bc])
        m["xfT"] = np.ascontiguousarray(xb["xfT"][:, c * Bc:(c + 1) * Bc])
        in_maps.append(m)
    res = run_bass_kernel_spmd(nc, in_maps, list(range(NCORES)))
    outs = [res.results[c]["outT"] for c in range(NCORES)]
    full = np.concatenate([o.T for o in outs], axis=0)
    return np.ascontiguousarray(full.astype(np.float32))
